# revision 17
# baseline (speedup 1.0000x reference)
"""ADM attention block (B=4, C=512, H=W=64) on 8 TRN2 NeuronCores.

Sharding: core = (b, half) = (core//2, core%2). Data-parallel over batch (4)
x query-halves (2). Zero collectives: each core computes the full QKV for its
batch sample (k, v needed in full anyway), then attention + output projection
for its half of the queries. The query half is selected purely on the host by
permuting the N axis of x so "my" queries are always columns 0:2048 (SPMD
cores run an identical graph; only inputs differ).

Per-core device algorithm:
  phase 0: weight-norm w = g * v / ||v_row|| folded as a column scale of v^T
           (ones-matmul column sumsq -> s = g*rsqrt -> DMA roundtrip
           broadcast across partitions -> scale, writing bf16 weights).
  phase 1: qkvT[n,3C] = x^T w_qkv^T, 128 n-rows at a time (PSUM [128,3x512]);
           RMS over 3C is a free-axis ACT square+accum; 1/(sqrt(mean)+1e-4);
           v-section is normalized straight into persistent v_sb[j,c'] (the
           attention rhs needs exactly this layout -- no transpose); q,k
           sections are normalized to bf16 then PE-transposed to [c,n].
  phase 2: per 256-query i-slice: scoresT[j,i] = k^T q per j-chunk;
           ex = exp(scores * C^-0.5) on ACT straight from PSUM; h[i,c'] and
           den[i] accumulate over j in PSUM (den via ones-column matmuls);
           epilogue: h *= 2^-0.5/den, PE-transpose to [c',i], project through
           w_out^T, add the (host-prescaled) residual, DMA out.

All matmuls/transposes run in bf16 (1 cycle/row, FWL weight loads); PSUM
accumulation is fp32; RMS/softmax denominators stay fp32.
"""

import os
from contextlib import ExitStack

import numpy as np
import ml_dtypes

import concourse.bass as bass
import concourse.mybir as mybir
import concourse.tile as tile
from concourse.bass_utils import run_bass_kernel_spmd

B, C, N = 4, 512, 4096
NH = N // 2
P = 128
KC = C // P            # 4 c-chunks
NCH = N // P           # 32 n-chunks
QCH = NH // P          # 16 query chunks per core
O3 = 3 * C             # 1536
F32 = mybir.dt.float32
F32R = mybir.dt.float32r
BF16 = mybir.dt.bfloat16
ISL = 512              # query i-slice
NISL = NH // ISL       # 4 i-slices

LAST_RESULT = None

_TPB_ENGINES = (
    mybir.EngineType.PE,
    mybir.EngineType.Activation,
    mybir.EngineType.DVE,
    mybir.EngineType.Pool,
    mybir.EngineType.SP,
)


def _split_waits(nc):
    """walrus on this image rejects >1 sem-wait on a TPB instruction (f32r
    matmul LDW lowering; tail Drain etc). Hoist excess waits onto engine-local
    NoOps, each carrying one wait -- semantically identical, waits run in
    queue order before the instruction."""
    ctr = 0
    for fn in nc.m.functions:
        for blk in fn.blocks:
            new_insts = []
            for inst in blk.instructions:
                si = getattr(inst, "sync_info", None)
                eng = getattr(inst, "engine", None)
                if (
                    si is not None
                    and si.on_wait
                    and len(si.on_wait) > 1
                    and eng in _TPB_ENGINES
                ):
                    for sw in si.on_wait[:-1]:
                        ctr += 1
                        nop = mybir.InstNoOp(
                            name=f"wsplit-{ctr}", engine=eng, ins=[], outs=[],
                            sync_info=mybir.SyncInfo(on_wait=[sw], on_update=[]),
                        )
                        new_insts.append(nop)
                    inst.sync_info = mybir.SyncInfo(
                        on_wait=[si.on_wait[-1]], on_update=si.on_update,
                    )
                new_insts.append(inst)
            blk.instructions[:] = new_insts


def build_graph():
    nc = bass.Bass()

    x_bf_d = nc.declare_dram_parameter("x_bf", [C, N], BF16, isOutput=False)
    xt_nc = nc.declare_dram_parameter("xt_nc", [NH, C], F32, isOutput=False)
    wqkvT_d = nc.declare_dram_parameter("wqkvT", [C, O3], F32, isOutput=False)
    g_qkv_d = nc.declare_dram_parameter("g_qkv", [1, O3], F32, isOutput=False)
    woutT_d = nc.declare_dram_parameter("woutT", [C, C], F32, isOutput=False)
    g_out_d = nc.declare_dram_parameter("g_out", [1, C], F32, isOutput=False)
    ident_d = nc.declare_dram_parameter("ident", [P, P], BF16, isOutput=False)
    identf_d = nc.declare_dram_parameter("identf", [P, P], F32, isOutput=False)
    ones_d = nc.declare_dram_parameter("ones_col", [P, 1], F32, isOutput=False)
    out_d = nc.declare_dram_parameter("out", [NH, C], F32, isOutput=True)

    with tile.TileContext(nc) as tc, ExitStack() as ctx:
        singles = ctx.enter_context(tc.tile_pool(name="singles", bufs=1))

        wq_sb = singles.tile([P, KC, O3], F32)
        nc.sync.dma_start(out=wq_sb, in_=wqkvT_d[:, :].rearrange("(k p) o -> p k o", p=P))
        wo_sb = singles.tile([P, KC, C], F32)
        nc.sync.dma_start(out=wo_sb, in_=woutT_d[:, :].rearrange("(k p) o -> p k o", p=P))
        ident = singles.tile([P, P], BF16)
        nc.sync.dma_start(out=ident, in_=ident_d[:, :])
        identf = singles.tile([P, P], F32)
        nc.sync.dma_start(out=identf, in_=identf_d[:, :])
        ones_mat = singles.tile([P, P], BF16)
        nc.vector.memset(ones_mat, 1.0)
        ones_bf = singles.tile([P, 1], BF16)
        nc.vector.memset(ones_bf, 1.0)
        ones_f = singles.tile([P, 1], F32R)
        nc.sync.dma_start(out=ones_f, in_=ones_d[:, :].bitcast(F32R))
        g_sb = singles.tile([1, O3], F32)
        nc.sync.dma_start(out=g_sb, in_=g_qkv_d[:, :])
        go_sb = singles.tile([1, C], F32)
        nc.sync.dma_start(out=go_sb, in_=g_out_d[:, :])

        wq_bf = singles.tile([P, KC, O3], BF16)
        wo_bf = singles.tile([P, KC, C], BF16)

        # ---- phase 0: weight-norm scales -> bf16 weights ----
        def fold_weight_norm(w_sb, w_bf, g_ap, odim):
            with tc.tile_pool(name="wnorm", bufs=1) as wn, \
                 tc.tile_pool(name="wnps", bufs=1, space="PSUM") as wnps:
                wsq = wn.tile([P, KC, odim], F32R)
                for kc in range(KC):
                    nc.scalar.square(wsq[:, kc, :], w_sb[:, kc, :])
                s_sb = wn.tile([1, odim], F32)
                tc.strict_bb_all_engine_barrier()
                for os_ in range(odim // 512):
                    n2 = wnps.tile([1, 512], F32)
                    for kc in range(KC):
                        nc.tensor.matmul(
                            n2,
                            lhsT=ones_f,
                            rhs=wsq[:, kc, os_ * 512:(os_ + 1) * 512],
                            start=(kc == 0), stop=(kc == KC - 1),
                        )
                    nc.scalar.copy(out=s_sb[:, os_ * 512:(os_ + 1) * 512], in_=n2)
                nc.scalar.sqrt(s_sb, s_sb)
                nc.vector.reciprocal(s_sb, s_sb)
                nc.vector.tensor_mul(s_sb, s_sb, g_ap)
                # broadcast across partitions via a DRAM-pool roundtrip
                with tc.tile_pool(name="sdr", bufs=1, space="DRAM") as drp:
                    sd = drp.tile([1, odim], F32)
                    nc.sync.dma_start(out=sd, in_=s_sb[0:1, :])
                    sbc = wn.tile([P, odim], F32)
                    sap = sd[0, :]
                    bcast = bass.AP(tensor=sap.tensor, offset=sap.offset,
                                    ap=[[0, P]] + list(sap.ap))
                    nc.sync.dma_start(out=sbc, in_=bcast)
                for kc in range(KC):
                    nc.vector.tensor_mul(w_bf[:, kc, :], w_sb[:, kc, :], sbc)

        fold_weight_norm(wq_sb, wq_bf, g_sb, O3)
        fold_weight_norm(wo_sb, wo_bf, go_sb, C)

        # ---- persistent attention operands ----
        tc.strict_bb_all_engine_barrier()
        big = ctx.enter_context(tc.tile_pool(name="big", bufs=1))
        k_a = big.tile([P, KC, NH], BF16)       # k_hat, [c-chunk][n<2048]
        k_b = big.tile([P, KC, NH], BF16)       # k_hat, [c-chunk][n>=2048]
        q_sb = big.tile([P, KC, NH], BF16)      # q_hat, [c-chunk][i]
        v_a = big.tile([P, NCH // 2, C], BF16)  # v_hat^T, [j<16][c']
        v_b = big.tile([P, NCH // 2, C], BF16)  # v_hat^T, [j>=16][c']

        # ---- phase 1: qkv projection + rms + q,k transposes ----
        x_re = x_bf_d[:, :].rearrange("(k p) n -> p k n", p=P)
        with tc.tile_pool(name="xp", bufs=3) as xpool, \
             tc.tile_pool(name="qkvps", bufs=2, space="PSUM") as qkvps, \
             tc.tile_pool(name="tpps", bufs=2, space="PSUM") as tpps, \
             tc.tile_pool(name="sqp", bufs=2) as sqp, \
             tc.tile_pool(name="qnp", bufs=2) as qnp, \
             tc.tile_pool(name="rp", bufs=4) as rp:
            for nch in range(NCH):
                x_sb = xpool.tile([P, KC, P], BF16)
                nc.sync.dma_start(out=x_sb, in_=x_re[:, :, nch * P:(nch + 1) * P])
                ps = qkvps.tile([P, 3, 512], F32)
                for os_ in range(3):
                    for kc in range(KC):
                        nc.tensor.matmul(
                            ps[:, os_, :],
                            lhsT=x_sb[:, kc, :],
                            rhs=wq_bf[:, kc, os_ * 512:(os_ + 1) * 512],
                            start=(kc == 0), stop=(kc == KC - 1),
                        )
                sq = sqp.tile([P, 3, 512], F32)
                ssum = rp.tile([P, 1], F32)
                nc.scalar.activation(out=sq, in_=ps,
                                     func=mybir.ActivationFunctionType.Square,
                                     accum_out=ssum)
                r = rp.tile([P, 1], F32)
                nc.scalar.activation(out=r, in_=ssum,
                                     func=mybir.ActivationFunctionType.Sqrt,
                                     scale=1.0 / O3)
                nc.vector.tensor_scalar_add(r, r, 1e-4)
                nc.vector.reciprocal(r, r)
                # v-section normalized straight into its attention layout
                v_half = v_a if nch < NCH // 2 else v_b
                nc.vector.tensor_scalar_mul(v_half[:, nch % (NCH // 2), :], ps[:, 2, :], r)
                # q,k sections -> bf16 -> PE transpose to [c, n]
                qn = qnp.tile([P, 2, 512], BF16)
                nc.vector.tensor_scalar_mul(qn, ps[:, 0:2, :], r)
                k_half = k_a if nch < QCH else k_b
                kcol = (nch % QCH) * P
                for cc in range(KC):
                    tp = tpps.tile([P, P], BF16, tag="tp")
                    nc.tensor.transpose(out=tp, in_=qn[:, 1, cc * P:(cc + 1) * P], identity=ident)
                    nc.vector.tensor_copy(out=k_half[:, cc, kcol:kcol + P], in_=tp)
                if nch < QCH:
                    for cc in range(KC):
                        tp = tpps.tile([P, P], BF16, tag="tp")
                        nc.tensor.transpose(out=tp, in_=qn[:, 0, cc * P:(cc + 1) * P], identity=ident)
                        nc.vector.tensor_copy(out=q_sb[:, cc, nch * P:(nch + 1) * P], in_=tp)

        # ---- phase 2: attention + projection ----
        with tc.tile_pool(name="scps", bufs=2, space="PSUM") as scps, \
             tc.tile_pool(name="hps", bufs=1, space="PSUM") as hps, \
             tc.tile_pool(name="dps", bufs=1, space="PSUM") as dps, \
             tc.tile_pool(name="tp2", bufs=1, space="PSUM") as tp2p, \
             tc.tile_pool(name="exp", bufs=3) as expp, \
             tc.tile_pool(name="hn", bufs=2) as hnp, \
             tc.tile_pool(name="hcn", bufs=2) as hcnp, \
             tc.tile_pool(name="xtp", bufs=2) as xtp, \
             tc.tile_pool(name="outp", bufs=2) as outp, \
             tc.tile_pool(name="rp2", bufs=8) as rp2:
            for isl in range(NISL):
                h_ps = hps.tile([P, 4, 512], F32, tag="hslot")
                den_ps = dps.tile([P, ISL], F32)
                for j in range(NCH):
                    k_half = k_a if j < QCH else k_b
                    v_half = v_a if j < NCH // 2 else v_b
                    kcol = (j % QCH) * P
                    sc = scps.tile([P, ISL], F32, tag="sc")
                    for cc in range(KC):
                        nc.tensor.matmul(
                            sc,
                            lhsT=k_half[:, cc, kcol:kcol + P],
                            rhs=q_sb[:, cc, isl * ISL:(isl + 1) * ISL],
                            start=(cc == 0), stop=(cc == KC - 1),
                        )
                    ex = expp.tile([P, ISL], BF16)
                    nc.scalar.activation(out=ex, in_=sc,
                                         func=mybir.ActivationFunctionType.Exp,
                                         scale=float(C) ** -0.5)
                    # den: ones-stationary -> every partition row = den[i]
                    nc.tensor.matmul(
                        den_ps,
                        lhsT=ones_mat,
                        rhs=ex,
                        start=(j == 0), stop=(j == NCH - 1),
                    )
                    for a in range(4):
                        nc.tensor.matmul(
                            h_ps[:, a, :],
                            lhsT=ex[:, a * P:(a + 1) * P],
                            rhs=v_half[:, j % (NCH // 2), :],
                            start=(j == 0), stop=(j == NCH - 1),
                        )
                # --- epilogue for this i-slice ---
                dencp = hnp.tile([P, ISL], F32, tag="dencp")
                nc.vector.tensor_copy(out=dencp, in_=den_ps)
                rdens = []
                for a in range(4):
                    dtp = tp2p.tile([P, P], F32, tag="tp2")
                    nc.tensor.transpose(out=dtp, in_=dencp[:, a * P:(a + 1) * P], identity=identf)
                    rden = rp2.tile([P, 1], F32)
                    nc.vector.reciprocal(rden, dtp[:, 0:1])
                    nc.vector.tensor_scalar_mul(rden, rden, float(2.0 ** -0.5))
                    rdens.append(rden)
                hn = hnp.tile([P, 4, 512], BF16, tag="hn")
                for a in range(4):
                    nc.vector.tensor_copy(out=hn[:, a, :], in_=h_ps[:, a, :])
                po = hps.tile([P, 4, 512], F32, tag="hslot")
                for a in range(4):
                    ich = isl * 4 + a
                    hcn = hcnp.tile([P, KC, P], BF16)
                    for cc in range(KC):
                        tp = tp2p.tile([P, P], BF16, tag="tp2")
                        nc.tensor.transpose(out=tp, in_=hn[:, a, cc * P:(cc + 1) * P], identity=ident)
                        nc.vector.tensor_copy(out=hcn[:, cc, :], in_=tp)
                    for cc in range(KC):
                        nc.tensor.matmul(
                            po[:, a, :],
                            lhsT=hcn[:, cc, :],
                            rhs=wo_bf[:, cc, :],
                            start=(cc == 0), stop=(cc == KC - 1),
                        )
                    xt_sb = xtp.tile([P, C], F32)
                    nc.sync.dma_start(out=xt_sb, in_=xt_nc[ich * P:(ich + 1) * P, :])
                    ob = outp.tile([P, C], F32)
                    nc.vector.scalar_tensor_tensor(
                        out=ob, in0=po[:, a, :], scalar=rdens[a], in1=xt_sb,
                        op0=mybir.AluOpType.mult, op1=mybir.AluOpType.add,
                    )
                    nc.sync.dma_start(out=out_d[ich * P:(ich + 1) * P, :], in_=ob)

    _split_waits(nc)
    return nc


_GRAPH = None


def kernel(**inputs):
    global _GRAPH, LAST_RESULT
    x = np.ascontiguousarray(np.asarray(inputs["x"], dtype=np.float32))
    v_qkv = np.ascontiguousarray(np.asarray(inputs["v_qkv"], dtype=np.float32))
    g_qkv = np.ascontiguousarray(np.asarray(inputs["g_qkv"], dtype=np.float32))
    v_out = np.ascontiguousarray(np.asarray(inputs["v_out"], dtype=np.float32))
    g_out = np.ascontiguousarray(np.asarray(inputs["g_out"], dtype=np.float32))

    xt = x.reshape(B, C, N)
    wqkvT = np.ascontiguousarray(v_qkv.T)
    woutT = np.ascontiguousarray(v_out.T)
    ident = np.eye(P, dtype=ml_dtypes.bfloat16)
    g_qkv2 = np.ascontiguousarray(g_qkv.reshape(1, O3))
    g_out2 = np.ascontiguousarray(g_out.reshape(1, C))
    rsqrt2 = np.float32(2.0 ** -0.5)

    in_maps = []
    for core in range(8):
        b, h = core // 2, core % 2
        if h == 0:
            x_perm = xt[b]
        else:
            x_perm = np.concatenate([xt[b][:, NH:], xt[b][:, :NH]], axis=1)
        x_perm = np.ascontiguousarray(x_perm)
        in_maps.append({
            "x_bf": x_perm.astype(ml_dtypes.bfloat16),
            "xt_nc": np.ascontiguousarray(x_perm[:, :NH].T * rsqrt2),
            "wqkvT": wqkvT,
            "g_qkv": g_qkv2,
            "woutT": woutT,
            "g_out": g_out2,
            "ident": ident,
            "identf": np.eye(P, dtype=np.float32),
            "ones_col": np.ones((P, 1), np.float32),
        })

    if _GRAPH is None:
        _GRAPH = build_graph()

    res = run_bass_kernel_spmd(_GRAPH, in_maps, core_ids=list(range(8)))
    LAST_RESULT = res

    out = np.empty((B, C, N), np.float32)
    for core in range(8):
        b, h = core // 2, core % 2
        out[b][:, h * NH:(h + 1) * NH] = res.results[core]["out"].T
    return out.reshape(B, C, 64, 64)
